# revision 45
# baseline (speedup 1.0000x reference)
"""Trainium2 Bass kernel for nn_AudioSelfAttention (B=2, T=2048, C=1024, H=16).

Sharding: sequence-parallel over the 8 NeuronCores. Core i handles batch
i//4 and query-token slice (i%4)*512. Each core computes K/V for its full
batch locally (redundant within the 4-core batch group — measured collective
cost here, ~76us floor + ~47us/MB, makes the AllGather alternative slower
than recomputation), computes attention for its own 512 query rows over all
16 heads, and the output projection. No collectives; the full output is
assembled on the host from the 8 row-shards.

Compute dtype: bf16 matmul operands (fp32 matmul is 4 cycles/row on TRN2's
PE vs 1 for bf16), fp32 PSUM accumulation and softmax statistics.

Layouts: activations live in transposed [feature, token] space so every
matmul contracts along partitions. Attention scores are computed as
S^T = K^T-chunks x Q^T (kt on partitions), softmax-exp runs on ScalarE over
4-bank PSUM spans, and the softmax denominator falls out of the y-matmul by
augmenting V with a ones-column (M=65). Per-pair normalization uses a tiny
selector-matmul to broadcast 1/sum across partitions. The v-part and
proj biases are folded in exactly on the host (softmax rows sum to 1, so
they reduce to a constant row added to the output); q/k biases are applied
on-device in the PSUM->SBUF copies.
"""
import numpy as np

_CACHE = {}

B, T, C, H, D = 2, 2048, 1024, 16, 64
TQ = T * B // 8          # 512 query tokens per core
CC = C // 128            # 8 contraction chunks
NPAIR = H // 2           # 8 head pairs
NKT = T // 128           # 16 kt chunks


def _build_nc():
    import concourse.bacc as bacc
    import concourse.tile as tile
    import concourse.mybir as mybir

    f32 = mybir.dt.float32
    bf16 = mybir.dt.bfloat16
    Exp = mybir.ActivationFunctionType.Exp

    nc = bacc.Bacc(None, num_devices=8)
    xq = nc.declare_dram_parameter("xq", [C, TQ], bf16, isOutput=False)
    xt = nc.declare_dram_parameter("xt", [C, T], bf16, isOutput=False)
    # wqk: q/k weights pre-shuffled on host into jc-major contiguous blocks
    # wqk[jc, p, cc, j] = W_attn[cc*128+p, jc*128+j]  (jc 0..7 = q, 8..15 = k)
    wqk = nc.declare_dram_parameter("wqk", [16, 128, CC, 128], bf16, isOutput=False)
    wv_in = nc.declare_dram_parameter("wv_in", [C, C], bf16, isOutput=False)
    ba = nc.declare_dram_parameter("ba", [2 * C], f32, isOutput=False)
    wp = nc.declare_dram_parameter("wp", [C, C], bf16, isOutput=False)
    sel2 = nc.declare_dram_parameter("sel2", [2, 128], bf16, isOutput=False)
    out = nc.declare_dram_parameter("out", [TQ, C], f32, isOutput=True)

    with tile.TileContext(nc) as tc:
        with (
            tc.tile_pool(name="big", bufs=1) as big,
            tc.tile_pool(name="wst", bufs=3) as wst,
            tc.tile_pool(name="kpool", bufs=4) as kpool,
            tc.tile_pool(name="pexp", bufs=4) as pexp,
            tc.tile_pool(name="small", bufs=2) as small,
            tc.tile_pool(name="mmps", bufs=2, space="PSUM") as mmps,
            tc.tile_pool(name="spool", bufs=2, space="PSUM") as spool,
            tc.tile_pool(name="ypool", bufs=2, space="PSUM") as ypool,
        ):
            # ---- persistent SBUF tensors (DMA emission order = priority:
            # q-phase inputs first, proj weights last) ----
            xq_sb = big.tile([128, CC, TQ], bf16)
            nc.sync.dma_start(xq_sb[:], xq.rearrange("(c p) t -> p c t", p=128))
            ba_sb = big.tile([128, 16], f32)
            nc.sync.dma_start(ba_sb[:], ba.rearrange("(c p) -> p c", p=128))
            # wq shares its slot with the proj partial accumulator (disjoint lifetimes)
            wq_all = big.tile([128, CC, CC, 128], bf16, tag="scratch16")
            for jc in range(CC):
                nc.sync.dma_start(wq_all[:, jc, :, :], wqk[jc])

            # big loads on the second HWDGE queue (ScalarE, idle during lead-in)
            xt_sb = big.tile([128, CC, T], bf16)
            nc.scalar.dma_start(xt_sb[:], xt.rearrange("(c p) t -> p c t", p=128))
            wv_sb = big.tile([128, CC, C], bf16)
            nc.scalar.dma_start(wv_sb[:], wv_in.rearrange("(c p) d -> p c d", p=128))
            sel2_sb = big.tile([2, 128], bf16)
            nc.sync.dma_start(sel2_sb[:], sel2[:])
            wp_sb = big.tile([128, CC, C], bf16)

            q_sb = big.tile([128, CC, TQ], bf16)
            # v pair blocks padded to 144 cols (288B, 16B-aligned for both
            # head slices): [vA(64) | 1 | pad(7) | vB(64) | 1 | pad(7)].
            # One extra pad block lets M=128 stationary reads over-run (junk
            # cols only land in unread PSUM partitions 65..127).
            v_sb = big.tile([128, NKT, NPAIR + 1, 144], bf16)
            nc.vector.memset(
                v_sb.rearrange("p t r (h f) -> p t r h f", h=2)[:, :, :, :, 64:65], 1.0
            )
            yt_sb = big.tile([128, CC, TQ], bf16)
            yhat_sb = big.tile([128, CC, TQ], bf16)

            # ---- Q phase: q^T[j, tq] = W_q^T @ xq ----
            for jc in range(CC):
                q_ps = mmps.tile([128, TQ], f32, tag="mm")
                for cc in range(CC):
                    nc.tensor.matmul(q_ps[:], wq_all[:, jc, cc, :], xq_sb[:, cc, :],
                                     start=(cc == 0), stop=(cc == CC - 1))
                nc.vector.tensor_scalar_add(q_sb[:, jc, :], q_ps[:], ba_sb[:, jc:jc + 1])

            # ---- K chunks (pair-granular): k^T[j, t] for full batch ----
            def emit_k(jc):
                wk_t = wst.tile([128, CC, 128], bf16, tag="w", name=f"wk_{jc}")
                nc.sync.dma_start(wk_t[:], wqk[8 + jc])
                k_t = kpool.tile([128, 4, TQ], bf16, tag="kp", name=f"k_{jc}")
                for tt in range(4):
                    k_ps = mmps.tile([128, TQ], f32, tag="mm", name=f"kps_{jc}_{tt}")
                    for cc in range(CC):
                        nc.tensor.matmul(k_ps[:], wk_t[:, cc, :],
                                         xt_sb[:, cc, tt * TQ:(tt + 1) * TQ],
                                         start=(cc == 0), stop=(cc == CC - 1))
                    nc.vector.tensor_scalar_add(k_t[:, tt, :], k_ps[:], ba_sb[:, 8 + jc:9 + jc])
                return k_t

            ktiles = {0: emit_k(0), 1: emit_k(1)}

            # ---- V phase: v[t, d] native for full batch ----
            for tc_i in range(NKT):
                for dh in range(2):
                    v_ps = mmps.tile([128, TQ], f32, tag="mm", name=f"vps_{tc_i}_{dh}")
                    for cc in range(CC):
                        nc.tensor.matmul(v_ps[:], xt_sb[:, cc, tc_i * 128:(tc_i + 1) * 128],
                                         wv_sb[:, cc, dh * TQ:(dh + 1) * TQ],
                                         start=(cc == 0), stop=(cc == CC - 1))
                    nc.vector.tensor_copy(
                        v_sb[:, tc_i, 4 * dh:4 * dh + 4, :]
                        .rearrange("p r (h f) -> p r h f", h=2)[:, :, :, 0:64],
                        v_ps.rearrange("p (r h f) -> p r h f", r=4, h=2),
                    )

            # proj weights load during attention (needed only at the end)
            nc.sync.dma_start(wp_sb[:], wp.rearrange("(c p) d -> p c d", p=128))

            # proj partial accumulator (slot shared with wq_all, whose
            # lifetime ends after the Q phase)
            opart_sb = big.tile([128, CC, TQ], f32, tag="scratch16")

            # deferred per-pair normalization: emitted 3 chunks into the NEXT
            # pair so the sums->reciprocal chain never stalls the in-order PE
            # at a pair boundary
            s2_tiles = {}

            def emit_norm(p):
                s2 = s2_tiles.pop(p)
                r2f = small.tile([2, TQ], f32, tag="r2f", name=f"r2f_{p}")
                nc.vector.reciprocal_approx_fast(r2f[:], s2[:])
                r2b = small.tile([2, TQ], bf16, tag="r2b", name=f"r2b_{p}")
                nc.vector.tensor_copy(r2b[:], r2f[:])
                bc = mmps.tile([128, TQ], f32, tag="mm", name=f"bc_{p}")
                nc.tensor.matmul(bc[:], sel2_sb[:], r2b[:], start=True, stop=True)
                nc.vector.tensor_mul(yt_sb[0:64, p, :], yhat_sb[0:64, p, :], bc[0:64, :])
                nc.vector.tensor_mul(yt_sb[64:128, p, :], yhat_sb[64:128, p, :],
                                     bc[64:128, :])

            # ---- attention, one head-pair at a time ----
            # Pipeline per pair: per kt-chunk c emit S(c) -> exp(c) -> y(c-1),
            # with the next pair's K-chunk matmuls drip-fed 2 per chunk so the
            # PE always has exp-independent work while ScalarE runs.
            for p in range(NPAIR):
                kt_next = p + 2 if p + 2 < NPAIR else None
                knext_state = {}

                def emit_knext(ci, p=p, kt_next=kt_next, st=None):
                    # two accumulation matmuls of k(p+2) per kt chunk index ci
                    if kt_next is None:
                        return
                    st = knext_state
                    tt, ai = ci // 4, (ci % 4) * 2
                    if ai == 0:
                        st["wk"] = wst.tile([128, CC, 128], bf16, tag="w",
                                            name=f"wkn_{kt_next}_{tt}") if tt == 0 else st["wk"]
                        if tt == 0:
                            nc.sync.dma_start(st["wk"][:], wqk[8 + kt_next])
                            st["kt"] = kpool.tile([128, 4, TQ], bf16, tag="kp",
                                                  name=f"k_{kt_next}")
                        st["ps"] = mmps.tile([128, TQ], f32, tag="mm",
                                             name=f"kn_{kt_next}_{tt}")
                    for cc in (ai, ai + 1):
                        nc.tensor.matmul(st["ps"][:], st["wk"][:, cc, :],
                                         xt_sb[:, cc, tt * TQ:(tt + 1) * TQ],
                                         start=(cc == 0), stop=(cc == CC - 1))
                    if ai == 6:
                        nc.vector.tensor_scalar_add(st["kt"][:, tt, :], st["ps"][:],
                                                    ba_sb[:, 8 + kt_next:9 + kt_next])
                        if tt == 3:
                            ktiles[kt_next] = st["kt"]

                k_t = ktiles.pop(p)
                yA = ypool.tile([128, TQ], f32, tag="y", name=f"yA_{p}")
                yB = ypool.tile([128, TQ], f32, tag="y", name=f"yB_{p}")
                pe_tiles = {}

                def emit_y(c, p=p, yA=yA, yB=yB):
                    pe_t = pe_tiles.pop(c)
                    vflat = v_sb[:, c].rearrange("p r f -> p (r f)")
                    nc.tensor.matmul(yA[:], vflat[:, p * 144:p * 144 + 128], pe_t[:, 0:TQ],
                                     start=(c == 0), stop=(c == NKT - 1))
                    nc.tensor.matmul(yB[:], vflat[:, p * 144 + 72:p * 144 + 200],
                                     pe_t[:, TQ:2 * TQ],
                                     start=(c == 0), stop=(c == NKT - 1))

                # proj partials drip-fed during the last two pairs: tiles 0-3
                # accumulate cc 0..5 during pair 6 (which has no K drip and
                # spare PE time), tiles 4-7 accumulate cc 0..6 during pair 7
                def emit_projpart(c, p=p):
                    if p < NPAIR - 2 or c not in (4, 8, 12, 14):
                        return
                    last = p == NPAIR - 1
                    i = (4 if last else 0) + {4: 0, 8: 1, 12: 2, 14: 3}[c]
                    ncc = CC - 1 if last else CC - 2
                    tt, oh = i // 2, i % 2
                    pp_ps = mmps.tile([128, TQ], f32, tag="mm", name=f"pp_{tt}_{oh}")
                    for cc in range(ncc):
                        nc.tensor.matmul(pp_ps[:], yt_sb[:, cc, tt * 128:(tt + 1) * 128],
                                         wp_sb[:, cc, oh * TQ:(oh + 1) * TQ],
                                         start=(cc == 0), stop=(cc == ncc - 1))
                    nc.vector.tensor_copy(opart_sb[:, i, :], pp_ps[:])

                for c in range(NKT):
                    tt, off = c // 4, (c % 4) * 128
                    sp = spool.tile([128, 2 * TQ], f32, tag="s", name=f"s_{p}_{c}")
                    nc.tensor.matmul(sp[:, 0:TQ], k_t[0:64, tt, off:off + 128],
                                     q_sb[0:64, p, :], start=True, stop=True)
                    nc.tensor.matmul(sp[:, TQ:2 * TQ], k_t[64:128, tt, off:off + 128],
                                     q_sb[64:128, p, :], start=True, stop=True)
                    pe_t = pexp.tile([128, 2 * TQ], bf16, tag="pe", name=f"pe_{p}_{c}")
                    nc.scalar.activation(pe_t[:], sp[:], Exp, scale=0.125)
                    pe_tiles[c] = pe_t
                    if c >= 2:
                        emit_y(c - 2)
                    emit_knext(c)
                    emit_projpart(c)
                    if c == 2 and p >= 1:
                        emit_norm(p - 1)
                emit_y(NKT - 2)
                emit_y(NKT - 1)

                # drain: stash unnormalized y + sums, freeing the accumulators
                s2 = small.tile([2, TQ], f32, tag="s2", name=f"s2_{p}")
                stB = small.tile([1, TQ], f32, tag="stB", name=f"stB_{p}")
                nc.vector.tensor_copy(s2[0:1, :], yA[64:65, :])
                nc.vector.tensor_copy(stB[:], yB[64:65, :])
                nc.vector.tensor_copy(yhat_sb[0:64, p, :], yA[0:64, :])
                nc.vector.tensor_copy(yhat_sb[64:128, p, :], yB[0:64, :])
                nc.sync.dma_start(s2[1:2, :], stB[:])
                s2_tiles[p] = s2

            emit_norm(NPAIR - 1)

            # ---- output projection: final contraction chunk(s) + stashed partials ----
            for tt in range(4):
                for oh in range(2):
                    i = 2 * tt + oh
                    o_ps = mmps.tile([128, TQ], f32, tag="mm", name=f"ops_{tt}_{oh}")
                    if i < 4:
                        nc.tensor.matmul(o_ps[:], yt_sb[:, CC - 2, tt * 128:(tt + 1) * 128],
                                         wp_sb[:, CC - 2, oh * TQ:(oh + 1) * TQ],
                                         start=True, stop=False)
                    nc.tensor.matmul(o_ps[:], yt_sb[:, CC - 1, tt * 128:(tt + 1) * 128],
                                     wp_sb[:, CC - 1, oh * TQ:(oh + 1) * TQ],
                                     start=(i >= 4), stop=True)
                    o_sb = small.tile([128, TQ], f32, tag="osb", name=f"osb_{tt}_{oh}")
                    nc.vector.tensor_add(o_sb[:], o_ps[:], opart_sb[:, 2 * tt + oh, :])
                    dma_eng = nc.sync if (tt + oh) % 2 == 0 else nc.scalar
                    dma_eng.dma_start(out[tt * 128:(tt + 1) * 128, oh * TQ:(oh + 1) * TQ], o_sb[:])
    nc.compile()
    return nc


def _get_nc():
    if "nc" not in _CACHE:
        _CACHE["nc"] = _build_nc()
    return _CACHE["nc"]


def _in_maps(x, W_attn, b_attn, W_proj, b_proj):
    import ml_dtypes
    bf = ml_dtypes.bfloat16
    x = np.asarray(x, np.float32).reshape(B, T, C)
    W_attn = np.asarray(W_attn, np.float32)
    b_attn = np.asarray(b_attn, np.float32)
    W_proj = np.asarray(W_proj, np.float32)
    b_proj = np.asarray(b_proj, np.float32)

    xt_all = [np.ascontiguousarray(x[b_].T).astype(bf) for b_ in range(B)]
    # jc-major contiguous q/k weight blocks: wqk[jc, p, cc, j]
    wqk = np.ascontiguousarray(
        W_attn[:, :2 * C].reshape(CC, 128, 16, 128).transpose(2, 1, 0, 3)
    ).astype(bf)
    wv = np.ascontiguousarray(W_attn[:, 2 * C:]).astype(bf)
    wp = W_proj.astype(bf)
    ba = np.ascontiguousarray(b_attn[:2 * C])
    sel2 = np.zeros((2, 128), np.float32)
    sel2[0, 0:64] = 1.0
    sel2[1, 64:128] = 1.0
    sel2 = sel2.astype(bf)

    maps = []
    for i in range(8):
        b_, r = i // 4, i % 4
        maps.append({
            "xq": np.ascontiguousarray(xt_all[b_][:, r * TQ:(r + 1) * TQ]),
            "xt": xt_all[b_],
            "wqk": wqk, "wv_in": wv, "ba": ba, "wp": wp, "sel2": sel2,
        })
    return maps


def run(x, W_attn, b_attn, W_proj, b_proj, trace=False):
    from concourse.bass_utils import run_bass_kernel_spmd
    nc = _get_nc()
    maps = _in_maps(x, W_attn, b_attn, W_proj, b_proj)
    res = run_bass_kernel_spmd(nc, maps, list(range(8)), trace=trace)
    out = np.empty((B, T, C), np.float32)
    for i in range(8):
        b_, r = i // 4, i % 4
        out[b_, r * TQ:(r + 1) * TQ, :] = res.results[i]["out"]
    # v-bias and proj-bias fold: softmax rows sum to 1, so
    # P @ (V + 1 b_v^T) = P @ V + b_v  ->  out += b_v @ W_proj + b_proj  (exact)
    b_attn = np.asarray(b_attn, np.float32)
    b_proj = np.asarray(b_proj, np.float32)
    if b_attn[2 * C:].any() or b_proj.any():
        out += (b_attn[2 * C:] @ np.asarray(W_proj, np.float32) + b_proj).astype(np.float32)
    return out, res


def kernel(x, W_attn, b_attn, W_proj, b_proj):
    out, _ = run(x, W_attn, b_attn, W_proj, b_proj, trace=False)
    return out
